# revision 18
# baseline (speedup 1.0000x reference)
"""Trainium2 Bass kernel for nn_Attention (GQA + RoPE + softmax-n + causal).

Full inputs -> shard DP2(batch) x TP4(heads) across 8 cores -> gather+sum.

v2: all-fp16 matmul operands (fp32 PSUM accumulation), fused pipeline:
  prologue: load weights + x^T chunk 0, project chunk 0
  iter it in 0..3:
     issue x^T loads for chunk it+1
     attention for q-chunk it (kt bands <= it chunks)  [PE/Act/DVE]
     projection + rope for chunk it+1                  [PE/DVE filler work]
  per head: scores^T[k,q] -> exp (Act) -> AV accumulate (PE)
            esum += e (DVE); den = partition-reduce(esum)+1 (Pool);
            rec -> broadcast -> oc = out*rec (DVE)
  wo: out[q,d] = sum_hf oc[hf].T @ wo[hf]; fp16 staging -> DRAM

Host: out[b] = sum over 4 TP shards of fp32(out_partial).
"""
import sys
import numpy as np

sys.path.insert(0, "/opt/trn_rl_repo")

import concourse.bass as bass
import concourse.bacc as bacc
import concourse.mybir as mybir
import concourse.tile as tile
from concourse import bass_utils
from concourse._compat import with_exitstack

import os
F32 = mybir.dt.float32
F16 = (mybir.dt.float16 if os.environ.get("K_DT") == "f16"
       else mybir.dt.bfloat16)


def _np16():
    if os.environ.get("K_DT") == "f16":
        return np.float16
    import ml_dtypes
    return ml_dtypes.bfloat16


EXP = mybir.ActivationFunctionType.Exp

B, S, D = 2, 2048, 2048
N_HEADS, N_KV_HEADS, HD = 16, 8, 128
TP = 4                      # tensor-parallel ways (x DP2 over batch = 8 cores)
QF = 4 * HD                 # per-core q feature cols   (512)
KF = 2 * HD                 # per-core k/v feature cols (256)
NQT = S // 128              # 16 seq tiles
NQC = S // 512              # 4  q-chunks
ND = D // 128               # 16 contraction tiles

_CACHE = {}


def _build(bench_reps=None):
    nc = bacc.Bacc("TRN2", target_bir_lowering=False, debug=False)

    names = [("xT", [D, S], F16), ("wq", [D, QF], F16), ("wk", [D, KF], F16),
             ("wv", [D, KF], F16), ("wo", [QF, D], F16),
             ("c2", [128, S], F32), ("sn", [128, S], F32),
             ("tri", [128, 128], F16), ("ones128", [128, 1], F16)]
    kind = "Internal" if bench_reps else "ExternalInput"
    io = {n: nc.dram_tensor(n, sh, dt, kind=kind) for n, sh, dt in names}
    if bench_reps:
        io["dummy"] = nc.dram_tensor("bench_in", [128, 1], F32,
                                     kind="ExternalInput")
    io["out"] = nc.dram_tensor("out", [S, D], F16, kind="ExternalOutput")

    with tile.TileContext(nc) as tc:
        if bench_reps:
            # fill internal DRAM inputs with benign constants (avoid
            # garbage -> denormal/NaN timing artifacts)
            with tc.tile_pool(name="fillp", bufs=1) as fp:
                f3t = fp.tile([128, 2048], F32, tag="fill32")
                fht = fp.tile([128, 2048], F16, tag="fillh")
                nc.gpsimd.memset(f3t[:], 0.001)
                nc.gpsimd.memset(fht[:], 0.001)
                for n, sh, dt in names:
                    r, c = sh
                    for r0 in range(0, r, 128):
                        rr = min(128, r - r0)
                        for c0 in range(0, c, 2048):
                            cc = min(2048, c - c0)
                            srcap = fht[:rr, :cc] if dt == F16 else f3t[:rr, :cc]
                            nc.sync.dma_start(io[n][r0:r0 + rr, c0:c0 + cc],
                                              srcap)
        if bench_reps and bench_reps > 1:
            with tc.For_i(0, bench_reps, 1):
                _emit(tc, nc, io)
        else:
            _emit(tc, nc, io)
    nc.compile()
    return nc


@with_exitstack
def _emit(ctx, tc, nc, io):
    ts = bass.ts
    persist = ctx.enter_context(tc.tile_pool(name="persist", bufs=1))

    # ---- persistent SBUF tensors (live whole kernel) ----
    tri = persist.tile([128, 128], F16, tag="tri")
    c2 = persist.tile([128, S], F32, tag="c2")
    sn = persist.tile([128, S], F32, tag="sn")
    ones128 = persist.tile([128, 1], F16, tag="ones128")
    nc.gpsimd.dma_start(tri[:], io["tri"][:])
    nc.gpsimd.dma_start(c2[:], io["c2"][:])
    nc.gpsimd.dma_start(sn[:], io["sn"][:])
    nc.gpsimd.dma_start(ones128[:], io["ones128"][:])

    # rotated Q^T/K^T: 6 head-rows x 4 chunks, [128, 512] each (fp16)
    qkT = [[persist.tile([128, 512], F16, tag=f"qkT{f}_{c}",
                         name=f"qkT{f}_{c}") for c in range(NQC)]
           for f in range(6)]
    # V natural: 16 tiles [128 seq, KF]
    vnat = [persist.tile([128, KF], F16, tag=f"vnat{st}", name=f"vnat{st}")
            for st in range(NQT)]

    wp = ctx.enter_context(tc.tile_pool(name="wp", bufs=1))
    xtp = ctx.enter_context(tc.tile_pool(name="xtp", bufs=32))
    ropep = ctx.enter_context(tc.tile_pool(name="rope", bufs=4))
    ep = ctx.enter_context(tc.tile_pool(name="ep", bufs=8))
    esp = ctx.enter_context(tc.tile_pool(name="esp", bufs=2))
    finp = ctx.enter_context(tc.tile_pool(name="fin", bufs=4))
    ocp = ctx.enter_context(tc.tile_pool(name="ocp", bufs=8))
    osbp = ctx.enter_context(tc.tile_pool(name="osb", bufs=2))
    den_mode = os.environ.get("K_DEN", "pe2")
    npp = int(os.environ.get("K_PP", "1" if den_mode == "pe2" else "2"))
    nw3 = int(os.environ.get("K_W3", "1" if den_mode == "pe2" else "2"))
    pp = ctx.enter_context(tc.tile_pool(name="pp", bufs=npp, space="PSUM"))
    scps = ctx.enter_context(tc.tile_pool(name="scps", bufs=2, space="PSUM"))
    outps = ctx.enter_context(tc.tile_pool(name="outps", bufs=2, space="PSUM"))
    w3ps = ctx.enter_context(tc.tile_pool(name="w3ps", bufs=nw3, space="PSUM"))
    denps = (ctx.enter_context(tc.tile_pool(name="denps", bufs=2, space="PSUM"))
             if den_mode == "pe2" else None)

    wq_sb = [wp.tile([128, QF], F16, tag=f"wq{d}", name=f"wq{d}")
             for d in range(ND)]
    wk_sb = [wp.tile([128, KF], F16, tag=f"wk{d}", name=f"wk{d}")
             for d in range(ND)]
    wv_sb = [wp.tile([128, KF], F16, tag=f"wv{d}", name=f"wv{d}")
             for d in range(ND)]
    wo_sb = [wp.tile([128, D], F16, tag=f"wo{hf}", name=f"wo{hf}")
             for hf in range(4)]

    def load_xt(c):
        xt = []
        for d in range(ND):
            t = xtp.tile([128, 512], F16, tag="xt", name=f"xt{c}_{d}")
            nc.sync.dma_start(t[:], io["xT"][ts(d, 128), ts(c, 512)])
            xt.append(t)
        return xt

    def proj_chunk_gen(sc, xt):
        """Generator: one PE matmul per next(); trailing rope/copy ops are
        emitted together with the chain's last matmul."""
        cs = ts(sc, 512)
        for f in range(6):
            wt, fo = (wq_sb, f * 128) if f < 4 else (wk_sb, (f - 4) * 128)
            ps = pp.tile([128, 512], F32, tag="proj")
            for d in range(ND):
                nc.tensor.matmul(ps[:], wt[d][:, fo:fo + 128], xt[d][:],
                                 start=(d == 0), stop=(d == ND - 1))
                if d < ND - 1:
                    yield
            # rope (dest-partition indexed, swap-free):
            #  rot[p<64]  = ps[p]*cos - ps[p+64]*sin = ps[p]*c2 + ps[p+64]*sn
            #  rot[p>=64] = ps[p]*cos + ps[p-64]*sin
            a = ropep.tile([128, 512], F32, tag="ropeA")
            b = ropep.tile([128, 512], F32, tag="ropeB")
            nc.vector.tensor_mul(a[:], ps[:], c2[:, cs])
            nc.vector.tensor_mul(b[0:64, :], ps[64:128, :], sn[0:64, cs])
            nc.vector.tensor_mul(b[64:128, :], ps[0:64, :], sn[64:128, cs])
            nc.vector.tensor_add(qkT[f][sc][:], a[:], b[:])
            yield
        # V natural: out[s,hd] = sum_d xT[d,s] * wv[d,hd]
        for sub in range(4):
            st = sc * 4 + sub
            ps = pp.tile([128, KF], F32, tag="proj")
            for d in range(ND):
                nc.tensor.matmul(ps[:], xt[d][:, ts(sub, 128)], wv_sb[d][:],
                                 start=(d == 0), stop=(d == ND - 1))
                if d < ND - 1:
                    yield
            nc.vector.tensor_copy(vnat[st][:], ps[:])
            yield

    def wo_qc_gen(qc, oc):
        """Generator: one wo matmul per next(); copies/DMA ride along."""
        for sub in range(4):
            st = qc * 4 + sub
            o3 = osbp.tile([128, D], F16, tag="o3")
            for dc in range(4):
                ps3 = w3ps.tile([128, 512], F32, tag="wo3")
                for hf in range(4):
                    nc.tensor.matmul(ps3[:], oc[hf][:, ts(sub, 128)],
                                     wo_sb[hf][:, ts(dc, 512)],
                                     start=(hf == 0), stop=(hf == 3))
                    if hf < 3:
                        yield
                if os.environ.get("K_O3") == "mix" and dc % 2 == 1:
                    nc.scalar.copy(o3[:, ts(dc, 512)], ps3[:])
                else:
                    nc.vector.tensor_copy(o3[:, ts(dc, 512)], ps3[:])
                yield
            nc.sync.dma_start(io["out"][ts(st, 128), :], o3[:])

    def proj_chunk(sc, xt):
        for _ in proj_chunk_gen(sc, xt):
            pass

    LAG = int(os.environ.get("K_LAG", "3"))

    def attn_qc(qc, fillers=()):
        """Attention for q-chunk qc. AV matmuls lag scores by LAG kt-steps
        (e tiles buffer in SBUF); PE gaps between score/AV pairs are filled
        by pulling from `fillers` (generators emitting one matmul each)."""
        qs = qc * 512
        fill = list(fillers)

        def pull(n):
            k = 0
            while fill and k < n:
                try:
                    next(fill[0])
                    k += 1
                except StopIteration:
                    fill.pop(0)

        nsteps = 4 * 4 * (qc + 1) + 8       # kt-steps this qc (approx)
        supply = {0: 160, 1: 224, 2: 224, 3: 64}[qc]
        rate = max(1, (supply + nsteps - 1) // nsteps) if fill else 0
        oc = []
        for h in range(4):
            gkv = h // 2
            out_ps = outps.tile([128, 512], F32, tag="out")
            nkt = 4 * (qc + 1)
            if den_mode == "pe2":
                den_ps = denps.tile([1, 512], F32, tag="den")
                esum = None
            else:
                esum = esp.tile([128, 512], F16, tag="esum")
            pend = []                        # [(kt, off, e)]

            def av_emit():
                kt, off, e = pend.pop(0)
                nc.tensor.matmul(out_ps[:, off:],
                                 vnat[kt][:, gkv * 128:(gkv + 1) * 128],
                                 e[:, off:],
                                 start=(kt == 0), stop=(kt == nkt - 1))
                if den_mode == "pe2":
                    nc.tensor.matmul(den_ps[:, off:], ones128[:], e[:, off:],
                                     start=(kt == 0), stop=(kt == nkt - 1))
                elif kt == 0:
                    nc.vector.tensor_copy(esum[:], e[:])
                else:
                    nc.vector.tensor_add(esum[:, off:], esum[:, off:],
                                         e[:, off:])

            for kt in range(nkt):
                off = max(0, 128 * kt - qs)
                ck, ko = kt // 4, (kt % 4) * 128
                sc_ps = scps.tile([128, 512], F32, tag="sc")
                nc.tensor.matmul(sc_ps[:, off:], qkT[4 + gkv][ck][:, ko:ko + 128],
                                 qkT[h][qc][:, off:], start=True, stop=True)
                e = ep.tile([128, 512], F16, tag="e")
                nc.scalar.activation(e[:, off:], sc_ps[:, off:], EXP)
                if kt >= 4 * qc:          # diagonal 128-block: causal mask
                    nc.vector.tensor_mul(e[:, off:off + 128],
                                         e[:, off:off + 128], tri[:])
                pend.append((kt, off, e))
                pull(rate)
                if len(pend) > LAG:
                    av_emit()
            while pend:
                pull(rate)
                av_emit()
            # denominator: sum over k partitions, +1 phantom logit
            denp = finp.tile([1, 512], F32, tag="denp")
            if den_mode == "pe2":
                nc.vector.tensor_scalar_add(denp[:], den_ps[:], 1.0)
            elif den_mode == "pe":
                den_ps = w3ps.tile([1, 512], F32, tag="wo3")
                nc.tensor.matmul(den_ps[:], ones128[:], esum[:],
                                 start=True, stop=True)
                nc.vector.tensor_scalar_add(denp[:], den_ps[:], 1.0)
            else:
                denall = finp.tile([128, 512], F32, tag="denall")
                nc.gpsimd.partition_all_reduce(denall[:], esum[:], 128,
                                               bass.bass_isa.ReduceOp.add)
                nc.vector.tensor_scalar_add(denp[:], denall[0:1, :], 1.0)
            rec = finp.tile([1, 512], F32, tag="rec")
            with nc.allow_low_precision(reason="recip of denom"):
                nc.vector.reciprocal(rec[:], denp[:])
            bcs = finp.tile([128, 512], F32, tag="bcs")
            nc.gpsimd.partition_broadcast(bcs[:], rec[:])
            o = ocp.tile([128, 512], F16, tag="oc", name=f"oc{h}_{qc}")
            nc.vector.tensor_mul(o[:], out_ps[:], bcs[:])
            oc.append(o)
        # drain remaining fillers densely
        while fill:
            try:
                next(fill[0])
            except StopIteration:
                fill.pop(0)
        return oc

    # ---- prologue: weights + x^T chunk 0, interleaved for early start ----
    xt0 = []
    for d in range(ND):
        nc.sync.dma_start(wq_sb[d][:], io["wq"][ts(d, 128), :])
        t = xtp.tile([128, 512], F16, tag="xt", name=f"xt0_{d}")
        nc.scalar.dma_start(t[:], io["xT"][ts(d, 128), 0:512])
        xt0.append(t)
    for d in range(ND):
        nc.sync.dma_start(wk_sb[d][:], io["wk"][ts(d, 128), :])
        nc.sync.dma_start(wv_sb[d][:], io["wv"][ts(d, 128), :])
    for hf in range(4):
        nc.gpsimd.dma_start(wo_sb[hf][:], io["wo"][ts(hf, 128), :])

    def drain(gen):
        for _ in gen:
            pass

    ph = os.environ.get("K_PHASE", "all")
    if ph == "p1":
        proj_chunk(0, xt0)
        for c in range(1, NQC):
            proj_chunk(c, load_xt(c))
    elif ph == "p2":
        for f in range(6):
            for c in range(NQC):
                nc.gpsimd.memset(qkT[f][c][:], 0.001)
        for st in range(NQT):
            nc.gpsimd.memset(vnat[st][:], 0.001)
        oc_prev = None
        for it in range(NQC):
            fl = [wo_qc_gen(it - 1, oc_prev)] if oc_prev else []
            oc_prev = attn_qc(it, fl)
        drain(wo_qc_gen(NQC - 1, oc_prev))
    elif os.environ.get("K_FUSE", "1") == "1":
        proj_chunk(0, xt0)
        oc_prev = None
        for it in range(NQC):
            fl = []
            if it + 1 < NQC:
                fl.append(proj_chunk_gen(it + 1, load_xt(it + 1)))
            if oc_prev is not None:
                fl.append(wo_qc_gen(it - 1, oc_prev))
            oc_prev = attn_qc(it, fl)
        drain(wo_qc_gen(NQC - 1, oc_prev))
    else:
        proj_chunk(0, xt0)
        for c in range(1, NQC):
            proj_chunk(c, load_xt(c))
        oc_prev = None
        for it in range(NQC):
            fl = [wo_qc_gen(it - 1, oc_prev)] if oc_prev else []
            oc_prev = attn_qc(it, fl)
        drain(wo_qc_gen(NQC - 1, oc_prev))


def _host_prep(x, freqs_cos, freqs_sin, wq, wk, wv, wo):
    """Build the 8 per-core input maps."""
    # de-interleave perm within every 128-col head block: [0,2,..,126,1,3,..,127]
    p128 = np.concatenate([np.arange(0, 128, 2), np.arange(1, 128, 2)])
    permq = np.concatenate([hb * 128 + p128 for hb in range(N_HEADS)])
    permk = np.concatenate([hb * 128 + p128 for hb in range(N_KV_HEADS)])
    wq_p = (wq / np.sqrt(np.float32(HD)))[:, permq]
    wk_p = wk[:, permk]

    cosT = np.ascontiguousarray(freqs_cos.T)            # [64, S]
    sinT = np.ascontiguousarray(freqs_sin.T)
    c2 = np.concatenate([cosT, cosT], 0).astype(np.float32)   # [128, S]
    sn = np.concatenate([-sinT, sinT], 0).astype(np.float32)

    ii, jj = np.meshgrid(np.arange(128), np.arange(128), indexing="ij")
    tri = (ii <= jj).astype(_np16())                 # [k, q] allow k<=q

    common = {"c2": c2, "sn": sn, "tri": tri,
              "ones128": np.ones((128, 1), _np16())}
    in_maps = []
    for core in range(8):
        b, t = divmod(core, TP)
        in_maps.append({
            "xT": np.ascontiguousarray(x[b].T).astype(_np16()),
            "wq": np.ascontiguousarray(
                wq_p[:, t * QF:(t + 1) * QF]).astype(_np16()),
            "wk": np.ascontiguousarray(
                wk_p[:, t * KF:(t + 1) * KF]).astype(_np16()),
            "wv": np.ascontiguousarray(
                wv[:, t * KF:(t + 1) * KF]).astype(_np16()),
            "wo": np.ascontiguousarray(
                wo[t * QF:(t + 1) * QF, :]).astype(_np16()),
            **common,
        })
    return in_maps


def kernel(x, freqs_cos, freqs_sin, wq, wk, wv, wo, _trace=False):
    in_maps = _host_prep(np.asarray(x, np.float32),
                         np.asarray(freqs_cos, np.float32),
                         np.asarray(freqs_sin, np.float32),
                         np.asarray(wq, np.float32), np.asarray(wk, np.float32),
                         np.asarray(wv, np.float32), np.asarray(wo, np.float32))
    if "nc" not in _CACHE:
        _CACHE["nc"] = _build()
    res = bass_utils.run_bass_kernel_spmd(_CACHE["nc"], in_maps, list(range(8)),
                                          trace=_trace)
    _CACHE["last_result"] = res
    out = np.zeros((B, S, D), np.float32)
    for core in range(8):
        b = core // TP
        out[b] += res.results[core]["out"].astype(np.float32)
    return out
